# revision 15
# baseline (speedup 1.0000x reference)
import sys
from contextlib import ExitStack

import numpy as np

sys.path.insert(0, "/opt/trn_rl_repo")

import jax
import jax.numpy as jnp

try:
    jax.config.update("jax_compilation_cache_dir", "/tmp/bass_jax_cache")
    jax.config.update("jax_persistent_cache_min_compile_time_secs", 0.0)
    jax.config.update("jax_persistent_cache_min_entry_size_bytes", 0)
except Exception:
    pass

import concourse.bass as bass
import concourse.tile as tile
from concourse import bacc, mybir
from concourse.bass2jax import (
    _bass_exec_p,
    install_neuronx_cc_hook,
    partition_id_tensor,
)
from concourse.bass_utils import run_bass_kernel_spmd  # fallback path
from jax.experimental.shard_map import shard_map
from jax.sharding import Mesh, NamedSharding, PartitionSpec

EPS = 1e-8

N = 10000
D_IN = 12
E = N * D_IN
T = E * D_IN
K_R = 16
K_A = 8
HID = 64
OUT_D = 32
N24 = 24
F32R = 32                 # padded feature rows per s (24 real + 8 zero)
GAMMA = 8.0
NCORES = 8
ED = E // NCORES          # 15000 edges/core
NNC = N // NCORES         # 1250 nodes/core
NI = 15104                # gather idx count (118*128)
GT = 512                 # edges per compute tile
EDP = 15008               # ED padded to 32 for output transpose

# input section offsets (bytes) in per-core "xb"
XB_D = 0                    # [ED] f16 own-edge d (also the core's d slice)
XB_U = XB_D + 2 * ED        # [3,ED] f16 own-edge u, component-major
XB_RW = XB_U + 6 * ED       # [16,944] i16 wrapped row idx
XB_PRM = XB_RW + 2 * NI     # params f32
P_C16 = 0                   # [16,1] rc
P_B96 = 16                  # [96,1] 3x (b24 | 8 zeros ... -50 pad)
P_W1A = 112                 # [16,64]
P_W1BP = 1136               # [32,64] W1 rows 16..40 + 8 zero rows
P_B1 = 3184                 # [64,1]
P_W2 = 3248                 # [64,32]
P_WSEL = 5296               # [36,12]
P_WBLK = 5728               # [12,96] block-diag kst
P_TOT = 5728 + 1152
XB_TOT = XB_PRM + 4 * P_TOT

YBLK_Q = ED * OUT_D
YBLK = YBLK_Q + 4 * OUT_D
YTOT = NCORES * YBLK

F32 = mybir.dt.float32
F16 = mybir.dt.float16
I8 = mybir.dt.int8
U8 = mybir.dt.uint8
I16 = mybir.dt.int16


def build(sim_mode=False, silu=True):
    ACT = mybir.ActivationFunctionType.Silu if silu else mybir.ActivationFunctionType.Exp
    nc = bacc.Bacc('TRN2', target_bir_lowering=False, debug=False,
                   num_devices=(1 if sim_mode else NCORES))
    XB = nc.dram_tensor('xb', [XB_TOT], U8, kind='ExternalInput').ap()
    XD = XB[XB_D : XB_D + 2 * ED].bitcast(F16).unsqueeze(0)       # [1,ED]
    XU = XB[XB_U : XB_U + 6 * ED].bitcast(F16)                    # flat [3*ED]
    XRW = XB[XB_RW : XB_RW + 2 * NI].bitcast(I16)                 # flat [NI]
    PRM = XB[XB_PRM : XB_PRM + 4 * P_TOT].bitcast(F32)
    if sim_mode:
        DF = nc.dram_tensor('dfull', [E], F16, kind='ExternalInput').ap()
        UG = nc.dram_tensor('ug', [NCORES, 3, ED], F16, kind='ExternalInput').ap()
        Y = nc.dram_tensor('y', [YBLK], U8, kind='ExternalOutput').ap()
    else:
        DF = nc.dram_tensor('dfull', [E], F16, addr_space='Shared').ap()
        UG = nc.dram_tensor('ug', [NCORES, 3, ED], F16, addr_space='Shared').ap()
        Y = nc.dram_tensor('y', [YTOT], U8, kind='ExternalOutput').ap()
        YG = nc.dram_tensor('yg', [YTOT], U8, addr_space='Shared').ap()
    TAB = nc.dram_tensor('tab', [N, 128], F16).ap()
    YL = nc.dram_tensor('yl', [YBLK], U8).ap()
    DSL = nc.dram_tensor('dsl', [2 * ED], U8).ap()   # bounce: collectives
    USL = nc.dram_tensor('usl', [6 * ED], U8).ap()   # cannot read IO tensors

    with tile.TileContext(nc) as tc, ExitStack() as ctx:
        consts = ctx.enter_context(tc.tile_pool(name='consts', bufs=1))
        mid = ctx.enter_context(tc.tile_pool(name='mid', bufs=3))
        hp = ctx.enter_context(tc.tile_pool(name='hp', bufs=3))
        qtl = ctx.enter_context(tc.tile_pool(name='qtl', bufs=2))
        psa = ctx.enter_context(tc.tile_pool(name='psa', bufs=1, space=bass.MemorySpace.PSUM))
        ps0 = ctx.enter_context(tc.tile_pool(name='ps0', bufs=2, space=bass.MemorySpace.PSUM))
        ps1 = ctx.enter_context(tc.tile_pool(name='ps1', bufs=1, space=bass.MemorySpace.PSUM))
        pso = ctx.enter_context(tc.tile_pool(name='pso', bufs=1, space=bass.MemorySpace.PSUM))

        if not sim_mode:
            nc.gpsimd.dma_start(DSL[:], XB[XB_D : XB_D + 2 * ED])
            nc.gpsimd.dma_start(USL[:], XB[XB_U : XB_U + 6 * ED])
            nc.gpsimd.collective_compute(
                'AllGather', mybir.AluOpType.bypass,
                replica_groups=[list(range(NCORES))],
                ins=[DSL.rearrange('(a b) -> a b', b=1000)],
                outs=[DF.bitcast(U8).rearrange('(a b) -> a b', b=1000)],
            )
            nc.gpsimd.collective_compute(
                'AllGather', mybir.AluOpType.bypass,
                replica_groups=[list(range(NCORES))],
                ins=[USL.rearrange('(a b) -> a b', b=1000)],
                outs=[UG.rearrange('c x (n b) -> (c x n) b', b=1000).bitcast(U8)],
            )
        # ---- per-node table: cols 0..35 = u (12c+s), cols 64..75 = d ----
        nc.gpsimd.dma_start(TAB[:, 64:76], DF.rearrange('(n s) -> n s', s=D_IN))
        for cb in range(NCORES):
            nc.gpsimd.dma_start(
                TAB[cb * NNC : (cb + 1) * NNC, 0:36].rearrange(
                    'n (c s) -> n c s', s=D_IN
                ),
                UG[cb].rearrange('c (n s) -> n c s', s=D_IN),
            )

        c16t = consts.tile([K_R, 1], F32)
        nc.gpsimd.dma_start(c16t[:], PRM[P_C16:P_C16 + 16].rearrange('(p f) -> p f', p=16))
        b96t = consts.tile([96, 1], F32)
        nc.gpsimd.dma_start(b96t[:], PRM[P_B96:P_B96 + 96].rearrange('(p f) -> p f', p=96))
        w1at = consts.tile([K_R, HID], F32)
        nc.gpsimd.dma_start(w1at[:], PRM[P_W1A:P_W1A + 1024].rearrange('(p f) -> p f', p=16))
        w1bp = consts.tile([96, HID], F32)
        for sl_ in range(3):
            nc.gpsimd.dma_start(
                w1bp[32 * sl_ : 32 * (sl_ + 1), :],
                PRM[P_W1BP:P_W1BP + 2048].rearrange('(p f) -> p f', p=32),
            )
        b1t = consts.tile([HID, 1], F32)
        nc.gpsimd.dma_start(b1t[:], PRM[P_B1:P_B1 + 64].rearrange('(p f) -> p f', p=64))
        w2t = consts.tile([HID, OUT_D], F32)
        nc.gpsimd.dma_start(w2t[:], PRM[P_W2:P_W2 + 2048].rearrange('(p f) -> p f', p=64))
        wsel = consts.tile([36, D_IN], F32)
        nc.gpsimd.dma_start(wsel[:], PRM[P_WSEL:P_WSEL + 432].rearrange('(p f) -> p f', p=36))
        wblk = consts.tile([D_IN, 96], F32)
        nc.gpsimd.dma_start(wblk[:], PRM[P_WBLK:P_WBLK + 1152].rearrange('(p f) -> p f', p=12))

        # ---- per-tile gather + compute ----
        idx_sb = consts.tile([128, NI // 16], I16)
        for rep in range(8):  # idx wrapped in 16 partitions, replicated
            nc.gpsimd.dma_start(
                idx_sb[16 * rep : 16 * (rep + 1), :],
                XRW.rearrange('(p f) -> p f', p=16),
            )
        out_sb = consts.tile([OUT_D, ED], F32)

        gpool = ctx.enter_context(tc.tile_pool(name='gpool', bufs=3))

        def emit_tile(e0, g, ni):
            # gather G[p, 0:ni] = TAB[row[e0+i], p] (ni >= g, %128)
            Gt = gpool.tile([128, ni], F16)
            nc.gpsimd.dma_gather(
                Gt[:].rearrange('p (a f) -> p a f', a=1),
                TAB[:, :], idx_sb[:, e0 // 16 : (e0 + ni) // 16],
                ni, ni, 128, transpose=True,
            )
            gv = Gt[:, :g]
            # own-edge u broadcast rows 0..35 aligned with G's u rows
            ue2 = gpool.tile([36, g], F16)
            for c in range(3):
                nc.gpsimd.dma_start(
                    ue2[12 * c : 12 * (c + 1), :],
                    XU[c * ED + e0 : c * ED + e0 + g].unsqueeze(0).partition_broadcast(D_IN),
                )
            # per-edge dij RBF -> W1a contribution
            dbc = mid.tile([K_R, g], F16)
            nc.gpsimd.dma_start(dbc[:], XD[:, e0:e0 + g].partition_broadcast(K_R))
            dsub = mid.tile([K_R, g], F32)
            nc.vector.tensor_scalar_sub(dsub[:], dbc[:], c16t[:])
            fij = mid.tile([K_R, g], F32)
            nc.vector.tensor_mul(fij[:], dsub[:], dsub[:])
            fij2 = mid.tile([K_R, g], F32)
            nc.scalar.activation(fij2[:], fij[:], mybir.ActivationFunctionType.Exp, scale=-GAMMA)
            pa = psa.tile([HID, g], F32)
            nc.tensor.matmul(pa[:], w1at[:], fij2[:])
            ha = hp.tile([HID, g], F32)
            nc.scalar.copy(ha[:], pa[:])

            # u products rows 0..35, D via PE selection matmul
            p36 = mid.tile([36, g], F32)
            nc.vector.tensor_mul(p36[:], gv[0:36, :], ue2[:, :g])
            pD = ps0.tile([D_IN, g], F32)
            nc.tensor.matmul(pD[:], wsel[:], p36[:])
            Dsb = mid.tile([D_IN, g], F32)
            nc.scalar.copy(Dsb[:], pD[:])
            D2 = mid.tile([D_IN, g], F32)
            nc.vector.tensor_mul(D2[:], Dsb[:], Dsb[:])
            dikf = mid.tile([76, g], F32)
            nc.vector.tensor_copy(dikf[64:76, :], gv[64:76, :])
            dik2 = mid.tile([76, g], F32)
            nc.vector.tensor_mul(dik2[64:76, :], dikf[64:76, :], dikf[64:76, :])

            x12s = [mid.tile([D_IN, g], F32, name=f'x12_{i}') for i in range(2)]
            ft2s = [mid.tile([96, g], F32, name=f'ft2_{i}') for i in range(2)]
            p0s = [ps1.tile([96, g], F32, name=f'p0_{i}') for i in range(2)]
            p1s = [psa.tile([HID, g], F32, name=f'p1_{i}') for i in range(2)]
            hss = [hp.tile([HID, g], F32, name=f'hs_{i}') for i in range(2)]
            hhs = [hp.tile([HID, g], F32, name=f'hh_{i}') for i in range(2)]
            op = pso.tile([OUT_D, g], F32)
            for grp in range(4):
                s0 = 3 * grp
                x12 = x12s[grp % 2]
                nc.gpsimd.dma_start(x12[0:3, :], dikf[64 + s0 : 67 + s0, :])
                nc.gpsimd.dma_start(x12[3:6, :], dik2[64 + s0 : 67 + s0, :])
                nc.gpsimd.dma_start(x12[6:9, :], Dsb[s0 : s0 + 3, :])
                nc.gpsimd.dma_start(x12[9:12, :], D2[s0 : s0 + 3, :])
                p0g = p0s[grp % 2]
                nc.tensor.matmul(p0g[:], wblk[:], x12[:])
                ft2g = ft2s[grp % 2]
                nc.scalar.activation(ft2g[:], p0g[:], mybir.ActivationFunctionType.Exp, bias=b96t[:])
                for sl_ in range(3):
                    s = s0 + sl_
                    p1 = p1s[s % 2]
                    nc.tensor.matmul(p1[:], w1bp[32 * sl_ : 32 * (sl_ + 1), :], ft2g[32 * sl_ : 32 * (sl_ + 1), :])
                    hs = hss[s % 2]
                    nc.vector.tensor_add(hs[:], p1[:], ha[:])
                    h = hhs[s % 2]
                    nc.scalar.activation(h[:], hs[:], ACT, bias=b1t[:])
                    nc.tensor.matmul(op[:], w2t[:], h[:], start=(s == 0), stop=(s == D_IN - 1))
            nc.scalar.copy(out_sb[:, e0:e0 + g], op[:])

        for it in range(ED // GT):
            emit_tile(it * GT, GT, GT)
        rem = ED - (ED // GT) * GT
        if rem:
            emit_tile((ED // GT) * GT, rem, NI - (ED // GT) * GT)

        # ---- int8 quantize + chunked transpose + pack ----
        mx = consts.tile([OUT_D, 1], F32)
        nc.vector.tensor_reduce(mx[:], out_sb[:], axis=mybir.AxisListType.X, op=mybir.AluOpType.max)
        mn = consts.tile([OUT_D, 1], F32)
        nc.vector.tensor_reduce(mn[:], out_sb[:], axis=mybir.AxisListType.X, op=mybir.AluOpType.min)
        negmn = consts.tile([OUT_D, 1], F32)
        nc.vector.tensor_scalar_mul(negmn[:], mn[:], -1.0)
        amax = consts.tile([OUT_D, 1], F32)
        nc.vector.tensor_scalar(amax[:], mx[:], negmn[:], 1e-30, mybir.AluOpType.max, mybir.AluOpType.max)
        rec = consts.tile([OUT_D, 1], F32)
        nc.vector.reciprocal(rec[:], amax[:])
        sinv = consts.tile([OUT_D, 1], F32)
        nc.vector.tensor_scalar_mul(sinv[:], rec[:], 127.0)
        QL = YL[:YBLK_Q].bitcast(I8).rearrange('(e f) -> e f', f=OUT_D)
        blocks = [59, 59, 59, 59, 59, 59, 59, 56]
        b0 = 0
        for nbk in blocks:
            ec0 = b0 * 32
            ecn = min(nbk * 32, ED - ec0)
            cw = nbk * 32
            q16 = qtl.tile([OUT_D, cw], F16)
            if ecn < cw:
                nc.vector.memset(q16[:, ecn:], 1536.0)
            nc.vector.tensor_scalar(
                q16[:, :ecn], out_sb[:, ec0:ec0 + ecn], sinv[:], 1536.0,
                mybir.AluOpType.mult, mybir.AluOpType.add,
            )
            nc.vector.tensor_scalar_sub(q16[:], q16[:], 1536.0)
            qT = qtl.tile([OUT_D, cw], F16)
            nc.vector.transpose(qT[:], q16[:])
            qi8 = qtl.tile([OUT_D, cw], I8)
            nc.vector.tensor_copy(qi8[:], qT[:])
            nfull = ecn // 32
            nc.gpsimd.dma_start(
                QL[ec0:ec0 + nfull * 32, :].rearrange('(b p) f -> p b f', p=32),
                qi8[:].rearrange('p (b f) -> p b f', f=OUT_D)[:, :nfull, :],
            )
            if ecn > nfull * 32:
                nc.gpsimd.dma_start(
                    QL[ec0 + nfull * 32 : ec0 + ecn, :],
                    qi8[: ecn - nfull * 32, nfull * OUT_D : (nfull + 1) * OUT_D],
                )
            b0 += nbk
        nc.gpsimd.dma_start(YL[YBLK_Q:].bitcast(F32).rearrange('(p f) -> p f', p=OUT_D), amax[:])
        if sim_mode:
            nc.gpsimd.dma_start(Y[:], YL[:])
        else:
            nc.gpsimd.collective_compute(
                'AllGather', mybir.AluOpType.bypass,
                replica_groups=[list(range(NCORES))],
                ins=[YL.rearrange('(a b) -> a b', b=1408)],
                outs=[YG.rearrange('(a b) -> a b', b=1408)],
            )
            nc.gpsimd.dma_start(Y[:], YG[:])
    nc.compile()
    return nc


def make_prm(W1, b1, W2, rc, ac):
    prm = np.zeros(P_TOT, np.float32)
    cf24 = np.concatenate([rc, ac]).astype(np.float32)
    b24 = -GAMMA * cf24 * cf24
    b96 = np.full(96, -50.0, np.float32)
    for sl_ in range(3):
        b96[32 * sl_ : 32 * sl_ + 24] = b24
    prm[P_C16:P_C16 + 16] = rc
    prm[P_B96:P_B96 + 96] = b96
    prm[P_W1A:P_W1A + 1024] = W1[:K_R].reshape(-1)
    w1bp = np.zeros((32, 64), np.float32)
    w1bp[:24] = W1[K_R:]
    prm[P_W1BP:P_W1BP + 2048] = w1bp.reshape(-1)
    prm[P_B1:P_B1 + 64] = b1
    prm[P_W2:P_W2 + 2048] = W2.reshape(-1)
    wsel = np.zeros((36, 12), np.float32)
    for c in range(3):
        for s in range(12):
            wsel[12 * c + s, s] = 1.0
    prm[P_WSEL:P_WSEL + 432] = wsel.reshape(-1)
    kst = np.zeros((4, 24), np.float32)
    kst[0, :16] = 2.0 * GAMMA * rc
    kst[1, :16] = -GAMMA
    kst[2, 16:] = -2.0 * GAMMA * ac     # x = -D
    kst[3, 16:] = -GAMMA
    wblk = np.zeros((12, 96), np.float32)
    for r in range(4):
        for sl_ in range(3):
            wblk[r * 3 + sl_, 32 * sl_ : 32 * sl_ + 24] = kst[r]
    prm[P_WBLK:P_WBLK + 1152] = wblk.reshape(-1)
    return prm


def pack_core(dev, d16, u16, row, prm_u8):
    sl = slice(dev * ED, (dev + 1) * ED)
    xb = np.empty(XB_TOT, np.uint8)
    xb[XB_D:XB_D + 2 * ED] = d16[sl].view(np.uint8)
    xb[XB_U:XB_U + 6 * ED] = np.ascontiguousarray(u16[sl].T).view(np.uint8).reshape(-1)
    rw = np.zeros(NI, np.int16)
    rw[:ED] = row[sl].astype(np.int16)
    xb[XB_RW:XB_RW + 2 * NI] = np.ascontiguousarray(rw.reshape(NI // 16, 16).T).view(np.uint8).reshape(-1)
    xb[XB_PRM:] = prm_u8
    return xb


_PROG = None
_RUNNER = None
_YBUF = None            # device-resident donated output buffer chain
LAST_RESULTS = None
LAST_RUN_S = None


def _build_program():
    return build(sim_mode=False, silu=True)


def _get_program():
    global _PROG
    if _PROG is None:
        _PROG = _build_program()
    return _PROG


class _Runner:
    """Caches the jitted shard_map wrapper around the bass custom call so
    warm calls skip retracing/lowering (run_bass_kernel_spmd rebuilds the
    jit every call, which costs ~0.2s under axon)."""

    def __init__(self, nc):
        install_neuronx_cc_hook()
        self.nc = nc
        partition_name = (
            nc.partition_id_tensor.name if nc.partition_id_tensor else None
        )
        in_names, out_names, out_avals = [], [], []
        for alloc in nc.m.functions[0].allocations:
            if not isinstance(alloc, mybir.MemoryLocationSet):
                continue
            name = alloc.memorylocations[0].name
            if alloc.kind == "ExternalInput":
                if name != partition_name:
                    in_names.append(name)
            elif alloc.kind == "ExternalOutput":
                shape = tuple(alloc.tensor_shape)
                dtype = mybir.dt.np(alloc.dtype)
                out_names.append(name)
                out_avals.append(jax.core.ShapedArray(shape, dtype))
        n_params = len(in_names)
        n_outs = len(out_avals)
        in_names_full = in_names + out_names
        if partition_name is not None:
            in_names_full.append(partition_name)

        def _body(*args):
            operands = list(args)
            if partition_name is not None:
                operands.append(partition_id_tensor())
            outs = _bass_exec_p.bind(
                *operands,
                out_avals=tuple(out_avals),
                in_names=tuple(in_names_full),
                out_names=tuple(out_names),
                lowering_input_output_aliases=(),
                sim_require_finite=True,
                sim_require_nnan=True,
                nc=nc,
            )
            return tuple(outs)

        devices = jax.devices()[:NCORES]
        assert len(devices) == NCORES
        self.mesh = Mesh(np.asarray(devices), ("core",))
        self.shspec = NamedSharding(self.mesh, PartitionSpec("core"))
        self.sharded = jax.jit(
            shard_map(
                _body,
                mesh=self.mesh,
                in_specs=(PartitionSpec("core"),) * (n_params + n_outs),
                out_specs=(PartitionSpec("core"),) * n_outs,
                check_rep=False,
            ),
            donate_argnums=tuple(range(n_params, n_params + n_outs)),
            keep_unused=True,
        )
        # device-side zero creation: no host->device upload for the donated
        # output buffers (the kernel overwrites every output element)
        self._zeros = jax.jit(
            lambda: tuple(
                jnp.zeros((NCORES * a.shape[0], *a.shape[1:]), a.dtype)
                for a in out_avals
            ),
            out_shardings=(self.shspec,) * n_outs,
        )

    def dispatch(self, xbg: np.ndarray):
        """Async: returns (global_out, shard0) with the host copy started."""
        global _YBUF
        if _YBUF is None:
            _YBUF = self._zeros()
        ybuf, _YBUF = _YBUF, None  # consumed by donation below
        outs = self.sharded(xbg, *ybuf)
        # fetch only core 0's shard: the kernel AllGathers the full output
        # onto every core, so one shard == the whole result.
        shard0 = None
        for s in outs[0].addressable_shards:
            idx = s.index[0]
            if idx.start in (None, 0):
                shard0 = s
                break
        # start the device->host copy now so its RPC latency overlaps the
        # device execution (~100ms saved vs fetching after blocking)
        try:
            shard0.data.copy_to_host_async()
        except Exception:
            pass
        return outs, shard0

    def fetch(self, outs, shard0) -> np.ndarray:
        global _YBUF
        y0 = np.asarray(shard0.data)
        _YBUF = outs  # donate these buffers on the next call
        return y0


def _get_runner():
    global _RUNNER
    if _RUNNER is None:
        _RUNNER = _Runner(_get_program())
    return _RUNNER


def _decode(y0: np.ndarray) -> np.ndarray:
    """[YTOT] u8 -> [E, OUT_D] f32 (dequantize per-core per-channel int8)."""
    blk = y0.reshape(NCORES, YBLK)
    q = blk[:, :YBLK_Q].view(np.int8).reshape(NCORES, ED, OUT_D)
    amax = blk[:, YBLK_Q:].view(np.float32)[:, :OUT_D]
    s = amax * np.float32(1.0 / 127.0)
    out = np.multiply(q, s[:, None, :], dtype=np.float32)
    return out.reshape(E, OUT_D)


def _numpy_fallback(pos, W1, b1, W2, b2, rc, ac, e_e, i_e, j_e, k_e):
    rij = pos[j_e] - pos[i_e]
    rik = pos[k_e] - pos[i_e]
    dij = np.sqrt((rij * rij).sum(-1))
    dik = np.sqrt((rik * rik).sum(-1))
    cos = np.clip((rij * rik).sum(-1) / (dij * dik + EPS), -1.0, 1.0)
    feat = np.concatenate(
        [
            np.exp(-GAMMA * (dij[:, None] - rc[None, :]) ** 2),
            np.exp(-GAMMA * (dik[:, None] - rc[None, :]) ** 2),
            np.exp(-GAMMA * (cos[:, None] - ac[None, :]) ** 2),
        ],
        axis=-1,
    ).astype(np.float32)
    hpre = feat @ W1 + b1
    h = hpre / (1.0 + np.exp(-hpre))
    emb = h @ W2 + b2
    emb *= (k_e != j_e)[:, None].astype(np.float32)
    out = np.zeros((E, OUT_D), np.float32)
    np.add.at(out, e_e, emb)
    return out


def _structured(e_e, i_e, j_e, k_e, row):
    """Sampled check that the index tensors follow setup_inputs() structure."""
    if e_e.shape != (T,) or i_e.shape != (T,) or j_e.shape != (T,) or k_e.shape != (T,):
        return False
    if row.min() < 0 or row.max() >= N:
        return False
    s = np.arange(0, T, 17, dtype=np.int64)
    es = s // D_IN
    if not np.array_equal(e_e[s].astype(np.int64), es):
        return False
    if not np.array_equal(j_e[s].astype(np.int64), es // D_IN):
        return False
    if not np.array_equal(i_e[s].astype(np.int64), row[es]):
        return False
    if not np.array_equal(
        k_e[s].astype(np.int64), row[row[es] * D_IN + s % D_IN]
    ):
        return False
    return True


def kernel(**inputs) -> np.ndarray:
    global LAST_RUN_S, LAST_RESULTS, _YBUF
    pos = np.asarray(inputs["pos"], np.float32)
    W1 = np.asarray(inputs["W1"], np.float32)
    b1 = np.asarray(inputs["b1"], np.float32)
    W2 = np.asarray(inputs["W2"], np.float32)
    b2 = np.asarray(inputs["b2"], np.float32)
    rc = np.asarray(inputs["r_centers"], np.float32)
    ac = np.asarray(inputs["a_centers"], np.float32)
    e_e = np.asarray(inputs["e_e"])
    i_e = np.asarray(inputs["i_e"])
    j_e = np.asarray(inputs["j_e"])
    k_e = np.asarray(inputs["k_e"])

    row = np.ascontiguousarray(i_e[::D_IN]).astype(np.int64)  # edge source node
    if not _structured(e_e, i_e, j_e, k_e, row):
        return _numpy_fallback(pos, W1, b1, W2, b2, rc, ac, e_e, i_e, j_e, k_e)

    # Per-edge geometry on host (E values); the device AllGathers the
    # d/u slices, builds a per-node table, and hardware-gathers the
    # per-triplet dik/cos inputs itself.
    dvec = np.repeat(pos, D_IN, axis=0) - pos[row]     # pos[col]-pos[row], [E,3]
    d = np.sqrt(np.einsum("es,es->e", dvec, dvec))     # [E] f32
    u = dvec / np.maximum(d, 1e-30)[:, None]           # [E,3] unit vectors
    d16 = d.astype(np.float16)
    u16 = u.astype(np.float16)

    prm = make_prm(W1, b1, W2, rc, ac)
    prm_u8 = prm.view(np.uint8)
    xbg = np.empty(NCORES * XB_TOT, np.uint8)
    for dev in range(NCORES):
        xbg[dev * XB_TOT : (dev + 1) * XB_TOT] = pack_core(
            dev, d16, u16, row, prm_u8
        )
    import gc as _gc
    import time as _time

    # keep interpreter GC pauses out of the dispatch path
    _gc_was_enabled = _gc.isenabled()
    _gc.disable()
    _t0 = _time.time()
    fallback = None
    try:
        try:
            r = _get_runner()
            handle = r.dispatch(xbg)
        except Exception:
            _YBUF = None
            handle = None
        # overlap fixup precompute with device execution
        bad = np.flatnonzero(k_e == j_e)               # masked k==j triplets
        if bad.size:
            e_bad = bad // D_IN
            s_bad = bad % D_IN
            m_bad = row[e_bad] * D_IN + s_bad
            d_bad = d16[e_bad].astype(np.float32)
            f_ij = np.exp(-GAMMA * (d_bad[:, None] - rc[None, :]) ** 2)
            dik_b = d16[m_bad].astype(np.float32)
            f_k = np.exp(-GAMMA * (dik_b[:, None] - rc[None, :]) ** 2)
            Db = np.einsum(
                "tc,tc->t",
                u16[e_bad].astype(np.float32),
                u16[m_bad].astype(np.float32),
            )
            f_c = np.exp(-GAMMA * ((-Db)[:, None] - ac[None, :]) ** 2)
            hpre = (
                f_ij @ W1[:K_R]
                + f_k @ W1[K_R : 2 * K_R]
                + f_c @ W1[2 * K_R :]
                + b1
            )
            hb = hpre / (1.0 + np.exp(-hpre))
            corr = (hb @ W2).astype(np.float32)
        try:
            if handle is None:
                raise RuntimeError("dispatch failed")
            y0 = r.fetch(*handle)
        except Exception:
            # transient device errors recover on retry; if not, fall back to
            # run_bass_kernel_spmd, then to the (slow but correct) host path
            _YBUF = None
            try:
                handle = r.dispatch(xbg)
                y0 = r.fetch(*handle)
            except Exception:
                _YBUF = None
                try:
                    in_maps = [
                        {"xb": xbg[dev * XB_TOT : (dev + 1) * XB_TOT]}
                        for dev in range(NCORES)
                    ]
                    res = run_bass_kernel_spmd(
                        _get_program(), in_maps, list(range(NCORES))
                    )
                    y0 = res.results[0]["y"]
                except Exception:
                    fallback = _numpy_fallback(
                        pos, W1, b1, W2, b2, rc, ac, e_e, i_e, j_e, k_e
                    )
    finally:
        if _gc_was_enabled:
            _gc.enable()
    LAST_RUN_S = _time.time() - _t0
    LAST_RESULTS = None
    if fallback is not None:
        return fallback

    out = _decode(y0)

    # Masked (k==j) triplets: the device computes every triplet with its
    # real dik/cos values, so subtract the masked ones' full contribution
    # (recomputed on host from the same f16 geometry the device used).
    if bad.size:
        np.subtract.at(out, e_bad, corr)

    if b2.any():
        cnt = np.bincount(
            e_e, weights=(k_e != j_e).astype(np.float64), minlength=E
        )
        out = out + cnt[:, None].astype(np.float32) * b2[None, :]
    return out
